# revision 2
# baseline (speedup 1.0000x reference)
"""TRN2 Bass/Tile kernel for nn_DotProductAttention (softmax over the QUERY axis).

reference:
    scores  = einsum('bqd,bkd->bqk', q, k) / sqrt(64)
    weights = softmax(scores, axis=1)          # over q, NOT k!
    out     = einsum('bqk,bkd->bqd', weights, v)

Works with the transposed score matrix T = K @ Q^T ([k, q]): the softmax
reduction axis (q) is the free axis, and the normalizer Z[k] lives on the
contraction axis of the AV matmul so it folds into V (Vs = V / Z).

Sharding: B=16 batches, data-parallel over 8 cores => 2 batches per core,
packed into the two 64-partition halves of [128, *] tiles.

Per-core structure:
  phase A (DMA only, no engine time): q/k are cast to bf16 by SWDGE
    casting DMAs into (b,d)-packed SBUF staging, bounced to DRAM [s, 128],
    and transposed back by the DMA xbar into QT/KT [128 (b d), 2048 s].
  phase B: for each of 32 (k-chunk, batch) tiles, two [128, 1024] score
    subtiles (3-deep PSUM pool) -> exp. Exp is split across engines:
    ACT tiles use the activation table (accum_out gives Z for free), DVE
    tiles use a Schraudolph fast-exp (affine in f32, write the int16 bit
    pattern of bf16; ~2% rms) with the row-sum Z rebuilt by a GPSIMD
    half-add plus a DVE reduce. Per 2-chunk group: Z -> 1/Z (batched
    reciprocal), Vs = V/Z, and the q-half-0 AV accumulation runs in the
    2 spare PSUM banks, overlapped with B1.
  tail: AV for q-half 1 (dense), PSUM drains, 16 PE transposes of O^T,
    and the output DMAs.
"""

import math
from contextlib import ExitStack

import numpy as np

import concourse.bass as bass  # noqa: F401
import concourse.mybir as mybir
import concourse.tile as tile
from bass_rust import add_dep_helper
from concourse import bacc, bass_utils
from concourse.masks import make_identity

FP32 = mybir.dt.float32
BF16 = mybir.dt.bfloat16
I16 = mybir.dt.int16

N_CORES = 8
B_FULL = 16
BPC = B_FULL // N_CORES  # batches per core = 2
S = 2048
D = 64
NCH = S // 128  # 16 key chunks of 128
NT = NCH * BPC  # 32 (chunk, batch) tiles
SCALE = 1.0 / math.sqrt(D)

# Schraudolph fast-exp constants for a bf16 bit pattern:
#   bf16_bits(exp(x)) ~= trunc(x * 128/ln2 + (127*128 - 6 + 0.5))
A16 = 128.0 / math.log(2.0)
BIAS16 = 127.0 * 128.0 - 6.0 + 0.5

# engine assignment per tile (32 tiles): 'D' tiles use the DVE fast-exp
# (plus GPS+DVE for Z), the rest use ACT. Ratio 20:12 balances
# ACT ~2.14us/tile against DVE ~3.0us/tile + GPS 2.8us/tile.
DVE_TILE = {2, 5, 7}  # within each group of 8 tile indices


def tile_engine(t: int) -> str:
    return "D" if (t % 8) in DVE_TILE else "A"


def emit_kernel(ctx: ExitStack, tc, q, k, v, o, qbf_dram, kbf_dram):
    nc = tc.nc

    const_pool = ctx.enter_context(tc.tile_pool(name="const", bufs=1))
    big = ctx.enter_context(tc.tile_pool(name="big", bufs=1))
    # PSUM: 3 x [128,1024] score subtile buffers (6 banks) + pot (2 banks)
    ps = ctx.enter_context(tc.tile_pool(name="ps", bufs=3, space="PSUM"))
    pp = ctx.enter_context(tc.tile_pool(name="pp", bufs=1, space="PSUM"))

    ident = const_pool.tile([128, 128], FP32, name="ident")
    make_identity(nc, ident)
    zw = const_pool.tile([128, 128], BF16, name="zw")
    nc.vector.memset(zw[:], 0.0)

    # (b,d)-packed transposed operands: partitions 0:64 = batch0 d, 64:128 = b1.
    QT = big.tile([128, S], BF16, name="QT")
    KT = big.tile([128, S], BF16, name="KT")
    # bf16 staging in (m b d) column layout, s on partitions
    qbf = big.tile([128, S], BF16, name="qbf")
    kbf = big.tile([128, S], BF16, name="kbf")
    # V chunks [128 k, (t d)] f32 and Vs = V / Z (bf16); t = i*BPC + b
    V = big.tile([128, NT * D], FP32, name="V")
    Vs = big.tile([128, NT * D], BF16, name="Vs")
    # E[(t)*S :+ S] = exp(scores*SCALE): [128 k, 2048 q] bf16, fully resident
    E = big.tile([128, NT * S], BF16, name="E")
    Ei16 = E[:].bitcast(I16)
    # per (tile, half) raw sums, per-tile Z and 1/Z
    zc = big.tile([128, 2 * NT], FP32, name="zc")
    zs = big.tile([128, NT], FP32, name="zs")
    rz = big.tile([128, NT], FP32, name="rz")
    # gpsimd half-add scratch (rotating pair)
    Tg = big.tile([128, 2 * 512], FP32, name="Tg")
    # O^T staging ((b,d) packed on partitions, q on free), f32
    OT = big.tile([128, S], FP32, name="OT")
    # O in natural layout: column chunk m holds [q-tile m, (b d)]
    O_all = big.tile([128, S], FP32, name="O_all")

    # ---------------- phase A: DMA-only input transform ----------------
    # SWDGE casting DMAs: q/k f32 DRAM -> bf16 SBUF staging (b,d)-packed.
    cast_dmas = []
    for src, stg in ((q, qbf), (k, kbf)):
        for b in range(BPC):
            dma = nc.gpsimd.dma_start(
                stg[:].rearrange("p (m b d) -> p m b d", m=NCH, b=BPC, d=D)[
                    :, :, b, :
                ],
                src[b].rearrange("(m p) d -> p m d", p=128),
            )
            cast_dmas.append(dma)
    # bounce to DRAM [s, 128] and xbar-transpose back to [128 (b d), s].
    # q on the sync queue (critical path), k on the scalar queue.
    nc.sync.dma_start(
        qbf_dram.rearrange("(m p) c -> p m c", p=128),
        qbf[:].rearrange("p (m c) -> p m c", m=NCH),
    )
    nc.sync.dma_start_transpose(QT[:, 0:1024], qbf_dram[0:1024, :])
    nc.sync.dma_start_transpose(QT[:, 1024:S], qbf_dram[1024:S, :])
    nc.scalar.dma_start(
        kbf_dram.rearrange("(m p) c -> p m c", p=128),
        kbf[:].rearrange("p (m c) -> p m c", m=NCH),
    )
    nc.scalar.dma_start_transpose(KT[:, 0:256], kbf_dram[0:256, :])
    nc.scalar.dma_start_transpose(KT[:, 256:S], kbf_dram[256:S, :])
    # V load (f32), (i b d) column layout; keep behind the casts on SWDGE.
    for b in range(BPC):
        vdma = nc.gpsimd.dma_start(
            V[:].rearrange("p (i b d) -> p i b d", i=NCH, b=BPC)[:, :, b, :],
            v[b].rearrange("(i p) d -> p i d", p=128),
        )
        add_dep_helper(
            vdma.ins, cast_dmas[-1].ins, sync=True, reason="V behind q/k casts"
        )

    # pot for q-half 0: open every (b, j) region with a zeroing matmul so
    # the partition-sliced AV matmuls can accumulate with start=False.
    pot0 = pp.tile([128, 1024], FP32, tag="pot", name="pot0")
    zmm0 = []
    for j in range(2):
        zmm0.append(
            nc.tensor.matmul(
                pot0[:, j * 512 : (j + 1) * 512],
                lhsT=zw[:],
                rhs=QT[:, 0:512],
                start=True,
                stop=False,
                skip_group_check=True,
            )
        )

    # ---------------- phase B1: scores -> exp (+Z), AV half 0 --------------
    av_done = []  # (region -> last mm) bookkeeping for stop flags

    def emit_av(pot, zmm, t, h, stop_last):
        b = t % BPC
        for j in range(2):
            mm = nc.tensor.matmul(
                pot[b * 64 : (b + 1) * 64, j * 512 : (j + 1) * 512],
                lhsT=Vs[:, t * D : (t + 1) * D],
                rhs=E[:, t * S + h * 1024 + j * 512 : t * S + h * 1024 + (j + 1) * 512],
                start=False,
                stop=stop_last,
                skip_group_check=True,
            )
            if zmm is not None:
                add_dep_helper(
                    mm.ins,
                    zmm[j].ins,
                    sync=False,
                    reason="AV after bank-opening zero matmul",
                )

    for i in range(NCH):
        for b in range(BPC):
            t = i * BPC + b
            eng = tile_engine(t)
            for h in range(2):
                sct = ps.tile([128, 1024], FP32, tag="ps", name=f"sc{t}_{h}")
                for j in range(2):
                    nc.tensor.matmul(
                        sct[:, j * 512 : (j + 1) * 512],
                        lhsT=KT[b * 64 : (b + 1) * 64, i * 128 : (i + 1) * 128],
                        rhs=QT[
                            b * 64 : (b + 1) * 64,
                            h * 1024 + j * 512 : h * 1024 + (j + 1) * 512,
                        ],
                        start=True,
                        stop=True,
                    )
                eb = t * S + h * 1024
                if eng == "A":
                    nc.scalar.activation(
                        E[:, eb : eb + 1024],
                        sct[:],
                        mybir.ActivationFunctionType.Exp,
                        scale=SCALE,
                        accum_out=zc[:, t * 2 + h : t * 2 + h + 1],
                    )
                else:
                    nc.vector.tensor_scalar(
                        Ei16[:, eb : eb + 1024],
                        sct[:],
                        SCALE * A16,
                        BIAS16,
                        mybir.AluOpType.mult,
                        op1=mybir.AluOpType.add,
                    )
                    g = (t * 2 + h) % 2
                    nc.gpsimd.tensor_tensor(
                        Tg[:, g * 512 : (g + 1) * 512],
                        E[:, eb : eb + 512],
                        E[:, eb + 512 : eb + 1024],
                        mybir.AluOpType.add,
                    )
                    nc.vector.tensor_reduce(
                        zc[:, t * 2 + h : t * 2 + h + 1],
                        Tg[:, g * 512 : (g + 1) * 512],
                        mybir.AxisListType.X,
                        mybir.AluOpType.add,
                    )
        # after odd chunks: finish the 4-tile group (2 chunks)
        if i % 2 == 1:
            g4 = (i - 1) * BPC  # first tile of the group
            zcv = zc[:].rearrange("p (t u) -> p t u", u=2)
            nc.vector.tensor_tensor(
                zs[:, g4 : g4 + 4],
                zcv[:, g4 : g4 + 4, 0],
                zcv[:, g4 : g4 + 4, 1],
                mybir.AluOpType.add,
            )
            nc.vector.reciprocal(rz[:, g4 : g4 + 4], zs[:, g4 : g4 + 4])
            for t in range(g4, g4 + 4):
                # alternate the V scaling between ACT and DVE
                if t % 2 == 0:
                    nc.scalar.mul(
                        Vs[:, t * D : (t + 1) * D],
                        V[:, t * D : (t + 1) * D],
                        rz[:, t : t + 1],
                    )
                else:
                    nc.vector.tensor_scalar_mul(
                        Vs[:, t * D : (t + 1) * D],
                        V[:, t * D : (t + 1) * D],
                        rz[:, t : t + 1],
                    )
            for t in range(g4, g4 + 4):
                emit_av(pot0, zmm0, t, 0, stop_last=(t == NT - 1))

    # ---------------- tail: AV half 1, drains, transposes, stores ----------
    pot1 = ps.tile([128, 1024], FP32, tag="ps", name="pot1")
    zmm1 = []
    for j in range(2):
        zmm1.append(
            nc.tensor.matmul(
                pot1[:, j * 512 : (j + 1) * 512],
                lhsT=zw[:],
                rhs=QT[:, 0:512],
                start=True,
                stop=False,
                skip_group_check=True,
            )
        )
    for t in range(NT):
        emit_av(pot1, zmm1, t, 1, stop_last=(t == NT - 1))

    # drains: pot0 as soon as its last AV lands, pot1 after the tail AVs
    for c in range(2):
        nc.scalar.copy(OT[:, c * 512 : (c + 1) * 512], pot0[:, c * 512 : (c + 1) * 512])
    for c in range(2):
        nc.scalar.copy(
            OT[:, 1024 + c * 512 : 1024 + (c + 1) * 512],
            pot1[:, c * 512 : (c + 1) * 512],
        )

    o_view = O_all[:].rearrange("p (m b d) -> p m b d", m=NCH, b=BPC, d=D)
    for grp in range(4):
        for m in range(4 * grp, 4 * grp + 4):
            ptc = ps.tile([128, 128], FP32, tag="ps", name=f"ptc_{m}")
            nc.tensor.transpose(ptc[:], OT[:, m * 128 : (m + 1) * 128], ident[:])
            if m % 2 == 0:
                nc.vector.tensor_copy(O_all[:, m * 128 : (m + 1) * 128], ptc[:])
            else:
                nc.scalar.copy(O_all[:, m * 128 : (m + 1) * 128], ptc[:])
        for b in range(BPC):
            nc.sync.dma_start(
                o[b, 4 * grp * 128 : (4 * grp + 4) * 128, :].rearrange(
                    "(m p) d -> p m d", p=128
                ),
                o_view[:, 4 * grp : 4 * grp + 4, b, :],
            )


_CACHE: dict = {}


def build_program():
    if "nc" in _CACHE:
        return _CACHE["nc"]
    nc = bacc.Bacc("TRN2", target_bir_lowering=False, debug=False)
    q = nc.dram_tensor("q", [BPC, S, D], FP32, kind="ExternalInput").ap()
    k = nc.dram_tensor("k", [BPC, S, D], FP32, kind="ExternalInput").ap()
    v = nc.dram_tensor("v", [BPC, S, D], FP32, kind="ExternalInput").ap()
    o = nc.dram_tensor("o", [BPC, S, D], FP32, kind="ExternalOutput").ap()
    qbf_dram = nc.dram_tensor("qbf_dram", [S, 128], BF16, kind="Internal").ap()
    kbf_dram = nc.dram_tensor("kbf_dram", [S, 128], BF16, kind="Internal").ap()
    with tile.TileContext(nc) as tc:
        with ExitStack() as ctx:
            emit_kernel(ctx, tc, q, k, v, o, qbf_dram, kbf_dram)
    nc.compile()
    _CACHE["nc"] = nc
    return nc


def make_in_maps(q, k, v):
    q = np.ascontiguousarray(q, dtype=np.float32)
    k = np.ascontiguousarray(k, dtype=np.float32)
    v = np.ascontiguousarray(v, dtype=np.float32)
    assert q.shape == (B_FULL, S, D), q.shape
    return [
        {
            "q": np.ascontiguousarray(q[c * BPC : (c + 1) * BPC]),
            "k": np.ascontiguousarray(k[c * BPC : (c + 1) * BPC]),
            "v": np.ascontiguousarray(v[c * BPC : (c + 1) * BPC]),
        }
        for c in range(N_CORES)
    ]


def kernel(q, k, v, _trace=False):
    nc = build_program()
    in_maps = make_in_maps(q, k, v)
    res = bass_utils.run_bass_kernel_spmd(
        nc, in_maps, core_ids=list(range(N_CORES)), trace=_trace
    )
    out = np.concatenate([r["o"] for r in res.results], axis=0)
    if _trace:
        return out, res
    return out
